# revision 10
# baseline (speedup 1.0000x reference)
"""Differential attention (B=2, N=2048, D=1024, H=8, HEAD_DIM=128) on 8 trn2
NeuronCores. Head-parallel: core h computes head h end-to-end, then an
AllToAll re-shards heads -> token blocks for the output projection, so each
core emits one 512-token slice of the final output (no cross-core reduction).

Layout convention on device: activations are kept feature-major ("transposed",
[feature, token]) so that matmuls contract over the partition dim without any
on-chip transposition of x. The host supplies x pre-transposed and transposes
the output back.
"""

import numpy as np

import concourse.bass as bass
import concourse.mybir as mybir
import concourse.tile as tile
from concourse.bass_utils import run_bass_kernel_spmd
from concourse.masks import make_identity
from concourse.vector_clock import ScopedClock

# ---------------------------------------------------------------- constants
B, N, D = 2, 2048, 1024
H, HD = 8, 128
DQK = HD // 2
PROJ = H * HD
T = B * N  # 4096 flattened tokens
NCORES = 8
TBLK = T // NCORES  # 512 tokens per core for the output projection
LAMBDA_INIT = 0.8 - 0.6 * float(np.exp(-0.3 * 12))
SCALE = DQK ** -0.5
EPS = 1e-6

KB = N // 128  # 16 key chunks per batch
QB = N // 512  # 4 query blocks of 512 per batch

FP = mybir.dt.float32


# ------------------------------------------------- walrus drain workaround
# This container's walrus rejects Drain instructions carrying >1 sync wait
# ("Too many sync wait commands"). Split the TileContext tail drain into one
# Drain per wait condition.
def _split_waits(nc, inst, max_waits=1):
    si = inst.ins.sync_info
    if si is None:
        return
    waits = list(si.on_wait)
    if len(waits) <= max_waits:
        return
    si.on_wait = waits[:max_waits]
    for w in waits[max_waits:]:
        d2 = nc.sync.drain(fusable=False)
        si2 = d2.ins.sync_info
        if si2 is None:
            d2.ins.sync_info = mybir.SyncInfo(on_wait=[w], on_update=[])
        else:
            si2.on_wait = [w]


def _split_all_multiwaits(nc, max_waits=1):
    """walrus here allows only `max_waits` sync-wait per instruction. Hoist
    extra waits onto fresh NoOps inserted just before the instruction on the
    same engine (engines dispatch in order, so semantics are preserved)."""
    uid = 0
    for fn in nc.m.functions:
        for bb in fn.blocks:
            il = bb.instructions
            changed = False
            out = []
            for inst in il:
                si = inst.sync_info
                waits = list(si.on_wait) if si is not None else []
                if len(waits) > max_waits:
                    for w in waits[:-max_waits]:
                        ev = mybir.InstEventSemaphore(
                            name=f"waitsplit_{uid}",
                            sync_info=mybir.SyncInfo(on_wait=[w], on_update=[]),
                            engine=inst.engine,
                        )
                        uid += 1
                        out.append(ev)
                    si.on_wait = waits[-max_waits:]
                    if inst.sync_info is not si:
                        inst.sync_info = si
                    changed = True
                out.append(inst)
            if changed:
                bb.instructions = out


def _patched_drain_and_barrier(self, tick_clock, wait_clock):
    nc = self.nc
    drain_inst = nc.sync.drain(fusable=False)
    wait_clock.add_sem_waits(
        drain_inst.ins, ScopedClock({None: tick_clock.global_clock})
    )
    _split_waits(nc, drain_inst)
    nc.all_engine_barrier()
    assert self.sems is not None
    popped = nc._tile_sem_poison_stack.pop()
    assert popped is self._sem_poison
    nc.clear_and_free_semaphores(list(self.sems.allocated().values()))
    nc.all_engine_barrier()


tile.TileContext._drain_and_barrier = _patched_drain_and_barrier


# ---------------------------------------------------------------- program
def build_program(dbg=False):
    nc = bass.Bass(
        "TRN2",
        target_bir_lowering=False,
        debug=False,
        enable_asserts=True,
        num_devices=NCORES,
    )

    xT = nc.dram_tensor("xT", [D, T], FP, kind="ExternalInput")
    wq = nc.dram_tensor("wq", [D, HD], FP, kind="ExternalInput")
    wk = nc.dram_tensor("wk", [D, HD], FP, kind="ExternalInput")
    wv = nc.dram_tensor("wv", [D, HD], FP, kind="ExternalInput")
    wp = nc.dram_tensor("wp", [PROJ, D], FP, kind="ExternalInput")
    lam = nc.dram_tensor("lam", [128, 1], FP, kind="ExternalInput")
    yT = nc.dram_tensor("yT", [D, TBLK], FP, kind="ExternalOutput")
    if dbg:
        d_qT = nc.dram_tensor("d_qT", [128, T], FP, kind="ExternalOutput")
        d_kT = nc.dram_tensor("d_kT", [128, T], FP, kind="ExternalOutput")
        d_va = nc.dram_tensor("d_va", [128, B * KB, HD + 1], FP, kind="ExternalOutput")
        d_U = nc.dram_tensor("d_U", [B * QB * 4, 128, 2 * (HD + 1)], FP, kind="ExternalOutput")
        d_a2a = nc.dram_tensor("d_a2a", [NCORES, 128, TBLK], FP, kind="ExternalOutput")

    DC = D // 128  # 8 contraction chunks for the qkv projection

    with tile.TileContext(nc, num_cores=NCORES) as tc:
        with (
            tc.tile_pool(name="consts", bufs=1) as consts,
            tc.tile_pool(name="dram", bufs=1, space="DRAM") as dram,
        ):
            ident = consts.tile([128, 128], FP)
            make_identity(nc, ident)
            lam_sb = consts.tile([128, 1], FP)
            nc.sync.dma_start(lam_sb[:], lam[:])

            wq_sb = consts.tile([128, DC, HD], FP)
            wk_sb = consts.tile([128, DC, HD], FP)
            wv_sb = consts.tile([128, DC, HD], FP)
            for w_dram, w_sb in ((wq, wq_sb), (wk, wk_sb), (wv, wv_sb)):
                nc.sync.dma_start(w_sb[:], w_dram.rearrange("(c p) m -> p c m", p=128))
            wp_sb = consts.tile([128, H, D], FP)
            nc.sync.dma_start(wp_sb[:], wp.rearrange("(h p) m -> p h m", p=128))

            qT_sb = consts.tile([128, T], FP)
            kT_sb = consts.tile([128, T], FP)
            # v, per (batch, key-chunk): [key, head_dim] plus a ones column
            # (col 128) so the PV matmul also accumulates the softmax denom.
            va = consts.tile([128, B * KB, HD + 1], FP)
            nc.vector.memset(va[:, :, HD : HD + 1], 1.0)

            a2a_in = dram.tile([NCORES, 128, TBLK], FP)
            a2a_out = dram.tile([NCORES, 128, TBLK], FP)

            # ---------------- phase A: qkv projection (feature-major) ----
            with (
                tc.tile_pool(name="xa", bufs=2) as xa,
                tc.tile_pool(name="pa", bufs=2, space="PSUM") as pa,
                tc.tile_pool(name="sa", bufs=2) as sa,
            ):
                xT_view = xT.rearrange("(c p) t -> p c t", p=128)
                for tb in range(T // 512):
                    ts = slice(tb * 512, (tb + 1) * 512)
                    xx = xa.tile([128, DC, 512], FP)
                    nc.sync.dma_start(xx[:], xT_view[:, :, ts])

                    qps = pa.tile([128, 512], FP)
                    kps = pa.tile([128, 512], FP)
                    vps = pa.tile([128, 512], FP)
                    for c in range(DC):
                        f = dict(start=(c == 0), stop=(c == DC - 1))
                        nc.tensor.matmul(qps[:], wq_sb[:, c, :], xx[:, c, :], **f)
                        nc.tensor.matmul(kps[:], wk_sb[:, c, :], xx[:, c, :], **f)
                        nc.tensor.matmul(vps[:], wv_sb[:, c, :], xx[:, c, :], **f)
                    nc.vector.tensor_copy(qT_sb[:, ts], qps[:])
                    nc.vector.tensor_copy(kT_sb[:, ts], kps[:])

                    # v must be token-major for the PV matmul: PE-transpose
                    # 128x128 chunks of vT.
                    vT = sa.tile([128, 512], FP)
                    nc.scalar.copy(vT[:], vps[:])
                    b = tb // QB
                    for j in range(4):
                        kb = (tb % QB) * 4 + j
                        vtp = pa.tile([128, 128], FP, tag="vtp")
                        nc.tensor.transpose(
                            vtp[:], vT[:, j * 128 : (j + 1) * 128], ident[:]
                        )
                        nc.scalar.copy(va[:, b * KB + kb, 0:HD], vtp[:])

            # ---------------- phase B: differential attention ------------
            with (
                tc.tile_pool(name="ps", bufs=1, space="PSUM") as ps,
                tc.tile_pool(name="pu", bufs=1, space="PSUM") as pu,
                tc.tile_pool(name="pp", bufs=3) as pp,
                tc.tile_pool(name="se", bufs=2) as se,
                tc.tile_pool(name="so", bufs=4) as so,
            ):
                for b in range(B):
                    for qb in range(QB):
                        tb = b * QB + qb  # global 512-token block id
                        qs = slice(b * N + qb * 512, b * N + (qb + 1) * 512)
                        U = [
                            pu.tile([128, 2 * (HD + 1)], FP, tag="U", bufs=4,
                                    name=f"U_{tb}_{i}")
                            for i in range(4)
                        ]
                        for kb in range(KB):
                            ks = slice(b * N + kb * 128, b * N + (kb + 1) * 128)
                            s12 = ps.tile([128, 1024], FP, tag="s12", bufs=2)
                            # S^T tiles [key, query] for both q/k streams,
                            # row-packed on the PE (K=64 each).
                            nc.tensor.matmul(
                                s12[:, 0:512], kT_sb[0:64, ks], qT_sb[0:64, qs],
                                start=True, stop=True,
                            )
                            nc.tensor.matmul(
                                s12[:, 512:1024], kT_sb[64:128, ks], qT_sb[64:128, qs],
                                start=True, stop=True,
                            )
                            p12 = pp.tile([128, 1024], FP)
                            nc.scalar.activation(
                                p12[:], s12[:], mybir.ActivationFunctionType.Exp
                            )
                            vak = va[:, b * KB + kb, :]
                            for s in range(2):
                                for sub in range(4):
                                    # start=True clears has_written for the
                                    # whole PSUM bank, so only the very first
                                    # matmul touching this U bank may set it.
                                    nc.tensor.matmul(
                                        U[sub][:, s * (HD + 1) : (s + 1) * (HD + 1)],
                                        p12[:, s * 512 + sub * 128 : s * 512 + (sub + 1) * 128],
                                        vak,
                                        start=(kb == 0 and s == 0),
                                        stop=(kb == KB - 1 and s == 1),
                                    )
                        # epilogue: softmax normalize, differential combine,
                        # RMSNorm; emit transposed chunks into the A2A buffer.
                        for sub in range(4):
                            u = U[sub]
                            if dbg:
                                ud = se.tile([128, 2 * (HD + 1)], FP, tag="ud")
                                nc.vector.tensor_copy(ud[:], u[:])
                                nc.sync.dma_start(d_U[tb * 4 + sub], ud[:])
                            r1 = se.tile([128, 1], FP, tag="r1")
                            r2 = se.tile([128, 1], FP, tag="r2")
                            nc.vector.reciprocal(r1[:], u[:, HD : HD + 1])
                            nc.vector.reciprocal(r2[:], u[:, 2 * HD + 1 : 2 * HD + 2])
                            r2l = se.tile([128, 1], FP, tag="r2l")
                            nc.vector.tensor_mul(r2l[:], r2[:], lam_sb[:])
                            t1 = se.tile([128, 128], FP, tag="t1")
                            t2 = se.tile([128, 128], FP, tag="t2")
                            nc.vector.tensor_scalar_mul(t1[:], u[:, 0:HD], r1[:])
                            nc.vector.tensor_scalar_mul(
                                t2[:], u[:, HD + 1 : 2 * HD + 1], r2l[:]
                            )
                            od = se.tile([128, 128], FP, tag="od")
                            nc.vector.tensor_sub(od[:], t1[:], t2[:])
                            # ms = EPS + mean(od^2)
                            sq = se.tile([128, 128], FP, tag="sq")
                            nc.vector.tensor_mul(sq[:], od[:], od[:])
                            ssum = se.tile([128, 1], FP, tag="ssum")
                            nc.vector.tensor_reduce(
                                ssum[:], sq[:], mybir.AxisListType.X,
                                mybir.AluOpType.add,
                            )
                            ms = se.tile([128, 1], FP, tag="ms")
                            nc.vector.tensor_scalar(
                                ms[:], ssum[:], 1.0 / HD, EPS,
                                mybir.AluOpType.mult, mybir.AluOpType.add,
                            )
                            rt = se.tile([128, 1], FP, tag="rt")
                            nc.scalar.sqrt(rt[:], ms[:])
                            rs = se.tile([128, 1], FP, tag="rs")
                            nc.vector.reciprocal(rs[:], rt[:])
                            on = se.tile([128, 128], FP, tag="on")
                            nc.vector.tensor_scalar_mul(on[:], od[:], rs[:])
                            onT_ps = pu.tile([128, 2 * (HD + 1)], FP, tag="U", bufs=4)
                            nc.tensor.transpose(onT_ps[:, 0:128], on[:], ident[:])
                            onT = so.tile([128, 128], FP, tag="onT")
                            nc.scalar.copy(onT[:], onT_ps[:, 0:128])
                            nc.sync.dma_start(
                                a2a_in[tb, :, sub * 128 : (sub + 1) * 128], onT[:]
                            )

            # ---------------- phase C: A2A + output projection ------------
            if dbg:
                nc.sync.dma_start(d_qT[:], qT_sb[:])
                nc.sync.dma_start(d_kT[:], kT_sb[:])
                nc.sync.dma_start(d_va[:], va[:])
                nc.sync.dma_start(d_a2a[:], a2a_in[:])
            nc.gpsimd.collective_compute(
                "AllToAll",
                mybir.AluOpType.bypass,
                replica_groups=[list(range(NCORES))],
                ins=[a2a_in.opt()],
                outs=[a2a_out.opt()],
            )
            with (
                tc.tile_pool(name="sc", bufs=1) as sc,
                tc.tile_pool(name="pc", bufs=2, space="PSUM") as pc,
                tc.tile_pool(name="sy", bufs=2) as sy,
            ):
                aa = sc.tile([128, H, TBLK], FP)
                for hh in range(H):
                    nc.sync.dma_start(aa[:, hh, :], a2a_out[hh])
                for oc in range(D // 128):
                    yps = pc.tile([128, TBLK], FP)
                    for hh in range(H):
                        nc.tensor.matmul(
                            yps[:],
                            wp_sb[:, hh, oc * 128 : (oc + 1) * 128],
                            aa[:, hh, :],
                            start=(hh == 0),
                            stop=(hh == H - 1),
                        )
                    yo = sy.tile([128, TBLK], FP)
                    nc.vector.tensor_copy(yo[:], yps[:])
                    nc.sync.dma_start(yT[oc * 128 : (oc + 1) * 128, :], yo[:])

    _split_all_multiwaits(nc)
    return nc


_PROGRAM = None


def _get_program():
    global _PROGRAM
    if _PROGRAM is None:
        _PROGRAM = build_program()
    return _PROGRAM


# ---------------------------------------------------------------- host side
def _prep_in_maps(x, w_qkv, w_proj, lambda_q1, lambda_k1, lambda_q2, lambda_k2,
                  rms_weight):
    x = np.asarray(x, dtype=np.float32)
    w_qkv = np.asarray(w_qkv, dtype=np.float32)
    w_proj = np.asarray(w_proj, dtype=np.float32)
    xT = np.ascontiguousarray(x.reshape(T, D).T)
    lam_val = (
        float(np.exp(np.sum(np.asarray(lambda_q1, np.float64) * np.asarray(lambda_k1, np.float64))))
        - float(np.exp(np.sum(np.asarray(lambda_q2, np.float64) * np.asarray(lambda_k2, np.float64))))
        + LAMBDA_INIT
    )
    lam_arr = np.full((128, 1), lam_val, dtype=np.float32)
    # fold rms_weight and (1 - lambda_init) into the output projection rows
    rw = np.asarray(rms_weight, np.float32)
    wp_full = np.ascontiguousarray(
        w_proj * np.tile(rw, H)[:, None] * np.float32(1.0 - LAMBDA_INIT)
    )
    in_maps = []
    for h in range(NCORES):
        hs = slice(h * HD, (h + 1) * HD)
        in_maps.append(
            {
                "xT": xT,
                "wq": np.ascontiguousarray(w_qkv[:, hs]) * np.float32(SCALE),
                "wk": np.ascontiguousarray(w_qkv[:, PROJ + h * HD : PROJ + (h + 1) * HD]),
                "wv": np.ascontiguousarray(w_qkv[:, 2 * PROJ + h * HD : 2 * PROJ + (h + 1) * HD]),
                "wp": wp_full,
                "lam": lam_arr,
            }
        )
    return in_maps


def _assemble(results):
    y = np.empty((T, D), dtype=np.float32)
    for c in range(NCORES):
        y[c * TBLK : (c + 1) * TBLK, :] = results[c]["yT"].T
    return y.reshape(B, N, D)


def kernel(x, w_qkv, w_proj, lambda_q1, lambda_k1, lambda_q2, lambda_k2,
           rms_weight):
    nc = _get_program()
    in_maps = _prep_in_maps(
        x, w_qkv, w_proj, lambda_q1, lambda_k1, lambda_q2, lambda_k2, rms_weight
    )
    res = run_bass_kernel_spmd(nc, in_maps, list(range(NCORES)))
    return _assemble(res.results)
